# revision 2
# baseline (speedup 1.0000x reference)
"""Trainium2 Bass kernel for nn_FELDMSTM_7988639171122 (8 NeuronCores).

kernel(**inputs) takes the FULL inputs and returns the FULL output.
Sharding: node dim N=2000 split 250/core across 8 cores; batch B=8 kept whole.

Math (exact reformulation of the reference, verified offline):
  For sample s=(b,n) with x_s [L=96, D=32] (time-major):
    P_s = C6.T @ x_s                       [6, 32]
      where C6[t, m]   = cos(2*pi*f_m*t/L), f in {1,4,5}
            C6[t, 3+m] = -sin(2*pi*f_m*t/L)
    Z_s[m]   = P_s[m] @ U1p[n,m] - P_s[3+m] @ U2p[n,m]
    Z_s[3+m] = P_s[m] @ U2p[n,m] + P_s[3+m] @ U1p[n,m]
      where U1p/U2p [32,32] absorb Wq (left) and Wo (right) into the
      per-node complex Fourier weights W1 + i W2.
    res_s = M_res @ x_s + G @ Z_s          [96, 32]
      where M_res = I - A_ma (A_ma = edge-replicated moving average, k=25)
            G = M_res @ Cinv (Cinv = scaled inverse-DFT columns).
  Biases bq/bk/bv/bo are exact no-ops: the selected DFT modes of a
  constant are zero, and M_res @ 1 = 0 (A_ma rows sum to 1).

Device pipeline per node-tile of 16 nodes (all 8 b stacked):
  MM-A: 8 matmuls (lhsT = zero-padded C6 variant per b) -> P psum [48, C]
  PE-transpose P -> [128, 48*nq] (f16); per-node mixing as 12 tiny f16
  matmuls/node packed into 32x32 PE subarrays (tile_position);
  PE-transpose Z back; B1 (M_res.T, f32r) + B2 (padded G.T, f16)
  accumulate res psum [96, C]; evac; strided t-major DMA in/out.
"""

import numpy as np
import concourse.bass as bass
import concourse.bacc as bacc
import concourse.mybir as mybir
from concourse.tile import TileContext
from concourse.bass_utils import run_bass_kernel_spmd

F32 = mybir.dt.float32
F32R = mybir.dt.float32r
F16 = mybir.dt.float16

L, D, H, E, MODES, KAVG = 96, 32, 4, 8, (1, 4, 5), 25
NB = 8          # batch
NNODE = 250     # nodes per core
NCORES = 8
NT = 16         # nodes per device tile
TD = L * D      # 3072


def _host_constants():
    t = np.arange(L)
    th = 2 * np.pi * np.outer(t, np.array(MODES)) / L
    C6 = np.concatenate([np.cos(th), -np.sin(th)], axis=1)
    pad = (KAVG - 1) // 2
    A = np.zeros((L, L))
    for tt in range(L):
        for w in range(KAVG):
            A[tt, min(max(tt + w - pad, 0), L - 1)] += 1.0 / KAVG
    M_res = np.eye(L) - A
    Cinv = np.concatenate([(2.0 / L) * np.cos(th), -(2.0 / L) * np.sin(th)], axis=1)
    G = M_res @ Cinv
    return C6, M_res, G


def _host_node_weights(W1, W2, Wq, Wo):
    N = W1.shape[0]
    WoT = Wo.T.reshape(H, E, D)
    U1 = np.einsum("nheom,hod->nmhed", W1, WoT).reshape(N, 3, H * E, D)
    U2 = np.einsum("nheom,hod->nmhed", W2, WoT).reshape(N, 3, H * E, D)
    U1p = np.einsum("hd,nmhe->nmde", Wq.reshape(H * E, D), U1)
    U2p = np.einsum("hd,nmhe->nmde", Wq.reshape(H * E, D), U2)
    return U1p, U2p


def _pack_core_weights(W1c, W2c, Wq, Wo, C6, M_res, G):
    nl = W1c.shape[0]
    U1p, U2p = _host_node_weights(
        np.asarray(W1c, np.float64), np.asarray(W2c, np.float64),
        np.asarray(Wq, np.float64), np.asarray(Wo, np.float64))
    nq = (nl + 3) // 4
    uw = np.zeros((128, nq * 9 * 32), np.float32)
    for n in range(nl):
        q, g = n // 4, n % 4
        for m in range(3):
            base = ((q * 3 + m) * 3) * 32
            uw[32 * g:32 * g + 32, base:base + 32] = U1p[n, m]
            uw[32 * g:32 * g + 32, base + 32:base + 64] = U2p[n, m]
            uw[32 * g:32 * g + 32, base + 64:base + 96] = -U2p[n, m]
    cc48 = np.zeros((96, 8 * 48), np.float32)
    for b in range(8):
        cc48[:, 48 * b + 6 * b:48 * b + 6 * b + 6] = C6
    gt48 = np.zeros((48, 8 * 96), np.float32)
    for b in range(8):
        for c in range(6):
            gt48[8 * c + b, 96 * b:96 * (b + 1)] = G[:, c]
    return {
        "uw": uw.astype(np.float16),
        "cc48": cc48.astype(np.float32),
        "gt48": gt48.astype(np.float16),
        "mat": np.ascontiguousarray(M_res.T, np.float32),
        "id48": np.eye(48, dtype=np.float16),
        "id128": np.eye(128, dtype=np.float16),
    }


def _shard_inputs(x, W1, W2, Wq, Wo):
    C6, M_res, G = _host_constants()
    maps = []
    for c in range(NCORES):
        n0 = c * NNODE
        xc = np.ascontiguousarray(
            x[:, n0:n0 + NNODE].reshape(NB * NNODE, TD).astype(np.float32,
                                                              copy=False))
        m = {"xin": xc}
        m.update(_pack_core_weights(W1[n0:n0 + NNODE], W2[n0:n0 + NNODE],
                                    Wq, Wo, C6, M_res, G))
        maps.append(m)
    return maps


def _build_kernel():
    nl, nq = NNODE, (NNODE + 3) // 4
    nc = bacc.Bacc("TRN2", target_bir_lowering=False, debug=False,
                   num_devices=NCORES)
    xin = nc.dram_tensor("xin", [NB * nl, TD], F32R, kind="ExternalInput")
    uw = nc.dram_tensor("uw", [128, nq * 9 * 32], F16, kind="ExternalInput")
    cc48 = nc.dram_tensor("cc48", [96, 8 * 48], F32R, kind="ExternalInput")
    gt48 = nc.dram_tensor("gt48", [48, 8 * 96], F16, kind="ExternalInput")
    mat = nc.dram_tensor("mat", [96, 96], F32R, kind="ExternalInput")
    id48 = nc.dram_tensor("id48", [48, 48], F16, kind="ExternalInput")
    id128 = nc.dram_tensor("id128", [128, 128], F16, kind="ExternalInput")
    rout = nc.dram_tensor("rout", [NB * nl, TD], F32, kind="ExternalOutput")

    ntiles = (nl + NT - 1) // NT

    with TileContext(nc) as tc:
        with (
            tc.tile_pool(name="consts", bufs=1) as consts,
            tc.tile_pool(name="xp", bufs=12) as xp,
            tc.tile_pool(name="small", bufs=2) as small,
            tc.tile_pool(name="outp", bufs=8) as outp,
            tc.tile_pool(name="ps48p", bufs=1, space="PSUM") as ps48p,
            tc.tile_pool(name="ptp", bufs=1, space="PSUM") as ptp,
            tc.tile_pool(name="zqp", bufs=1, space="PSUM") as zqp,
            tc.tile_pool(name="ztp", bufs=1, space="PSUM") as ztp,
            tc.tile_pool(name="resp", bufs=3, space="PSUM") as resp,
        ):
            uw_sb = consts.tile([128, nq * 9 * 32], F16)
            nc.sync.dma_start(out=uw_sb[:], in_=uw.ap())
            cc_sb = consts.tile([96, 8 * 48], F32R)
            nc.sync.dma_start(out=cc_sb[:], in_=cc48.ap())
            gt_sb = consts.tile([48, 8 * 96], F16)
            nc.sync.dma_start(out=gt_sb[:], in_=gt48.ap())
            mat_sb = consts.tile([96, 96], F32R)
            nc.sync.dma_start(out=mat_sb[:], in_=mat.ap())
            id48_sb = consts.tile([48, 48], F16)
            nc.sync.dma_start(out=id48_sb[:], in_=id48.ap())
            id128_sb = consts.tile([128, 128], F16)
            nc.sync.dma_start(out=id128_sb[:], in_=id128.ap())

            for jt in range(ntiles):
                nt = min(NT, nl - jt * NT)
                C = nt * 32
                nqt = (nt + 3) // 4

                ps48 = ps48p.tile([48, 512], F32, tag="ps48")
                xts = []
                for b in range(NB):
                    s0 = b * nl + jt * NT
                    xt = xp.tile([96, 512], F32R, tag="xt")
                    src = bass.AP(tensor=xin, offset=s0 * TD,
                                  ap=[[32, 96], [TD, nt], [1, 32]])
                    nc.sync.dma_start(out=xt[:, :C], in_=src)
                    xts.append(xt)
                    nc.tensor.matmul(
                        ps48[:, :C], cc_sb[:, 48 * b:48 * (b + 1)],
                        xt[:, :C], start=(b == 0), stop=(b == NB - 1))

                ps_sb = small.tile([48, 512], F16, tag="ps_sb")
                nc.vector.tensor_copy(ps_sb[:, :C], ps48[:, :C])

                pt_ps = ptp.tile([128, 192], F16, tag="pt")
                pt_sb = small.tile([128, 192], F16, tag="pt_sb")
                for c in range(nqt):
                    w = min(128, C - 128 * c)
                    nc.tensor.transpose(
                        pt_ps[:w, 48 * c:48 * c + 48],
                        ps_sb[:, 128 * c:128 * c + w], id48_sb[:])
                    nc.vector.tensor_copy(
                        pt_sb[:w, 48 * c:48 * c + 48],
                        pt_ps[:w, 48 * c:48 * c + 48])

                zq_ps = zqp.tile([128, 192], F32, tag="zq")
                first_mix = True
                for nn in range(nt):
                    c, g = nn // 4, nn % 4
                    q = (jt * NT + nn) // 4
                    for m in range(3):
                        wbase = ((q * 3 + m) * 3) * 32
                        u1 = uw_sb[32 * g:32 * g + 32, wbase:wbase + 32]
                        u2 = uw_sb[32 * g:32 * g + 32, wbase + 32:wbase + 64]
                        u2n = uw_sb[32 * g:32 * g + 32, wbase + 64:wbase + 96]

                        def ptcols(cidx):
                            s = 48 * c + cidx
                            return pt_sb[32 * g:32 * g + 32,
                                         s:s + 6 * (NB - 1) + 1:6]

                        def zqcols(cp):
                            s = 48 * c + 8 * cp
                            return zq_ps[32 * g:32 * g + 32, s:s + NB]

                        tp = (32 * g, 32 * g)
                        nc.tensor.matmul(zqcols(m), u1, ptcols(m),
                                         start=(nn < 4 and m == 0), stop=False,
                                         tile_position=tp)
                        first_mix = False
                        nc.tensor.matmul(zqcols(m), u2n, ptcols(3 + m),
                                         start=False, stop=False,
                                         tile_position=tp)
                        nc.tensor.matmul(zqcols(3 + m), u2, ptcols(m),
                                         start=False, stop=False,
                                         tile_position=tp)
                        last = (nn == nt - 1) and (m == 2)
                        nc.tensor.matmul(zqcols(3 + m), u1, ptcols(3 + m),
                                         start=False, stop=last,
                                         tile_position=tp)

                zq_sb = small.tile([128, 192], F16, tag="zq_sb")
                nc.vector.tensor_copy(zq_sb[:, :48 * nqt], zq_ps[:, :48 * nqt])

                zt_ps = ztp.tile([48, 512], F16, tag="zt")
                zt_sb = small.tile([48, 512], F16, tag="zt_sb")
                for c in range(nqt):
                    w = min(128, C - 128 * c)
                    nc.tensor.transpose(
                        zt_ps[:, 128 * c:128 * c + w],
                        zq_sb[:w, 48 * c:48 * c + 48], id128_sb[:w, :w])
                    nc.vector.tensor_copy(
                        zt_sb[:, 128 * c:128 * c + w],
                        zt_ps[:, 128 * c:128 * c + w])

                for b in range(NB):
                    s0 = b * nl + jt * NT
                    res_ps = resp.tile([96, 512], F32, tag="res")
                    nc.tensor.matmul(res_ps[:, :C], mat_sb[:],
                                     xts[b][:, :C], start=True, stop=False)
                    nc.tensor.matmul(res_ps[:, :C],
                                     gt_sb[:, 96 * b:96 * (b + 1)],
                                     zt_sb[:, :C], start=False, stop=True)
                    out_sb = outp.tile([96, 512], F32, tag="out")
                    if b % 2 == 0:
                        nc.vector.tensor_copy(out_sb[:, :C], res_ps[:, :C])
                    else:
                        nc.scalar.copy(out=out_sb[:, :C], in_=res_ps[:, :C])
                    dst = bass.AP(tensor=rout, offset=s0 * TD,
                                  ap=[[32, 96], [TD, nt], [1, 32]])
                    nc.sync.dma_start(out=dst, in_=out_sb[:, :C])
    nc.compile()
    return nc


_NC_CACHE = None


def kernel(x, Wq, bq, Wk, bk, Wv, bv, Wo, bo, W1, W2):
    """Full inputs -> full output res [8, 2000, 96, 32] float32."""
    global _NC_CACHE
    x = np.asarray(x)
    maps = _shard_inputs(x, np.asarray(W1), np.asarray(W2),
                         np.asarray(Wq), np.asarray(Wo))
    if _NC_CACHE is None:
        _NC_CACHE = _build_kernel()
    res = run_bass_kernel_spmd(_NC_CACHE, maps, list(range(NCORES)))
    out = np.empty((NB, NCORES * NNODE, L, D), np.float32)
    for c in range(NCORES):
        rc = res.results[c]["rout"].reshape(NB, NNODE, L, D)
        out[:, c * NNODE:(c + 1) * NNODE] = rc
    return out
